# revision 24
# baseline (speedup 1.0000x reference)
"""Trainium2 Bass kernel for a pre-LN transformer encoder layer.

Model: D_MODEL=1024, N_HEADS=16, D_K=64, D_FF=4096, B=2, S=2048, fp32 I/O.

Sharding: fully data-parallel over 8 cores = (batch b, query-block j) with
512 query tokens per core.  Each core recomputes LN1/K/V for its full batch
element (no collectives), computes attention + FFN for its own 512 tokens,
and writes its [512, 1024] slice of the output.  Per-core inputs are rotated
so the core's own query block is always tokens [0:512) (attention is exactly
permutation-invariant over keys, so rotating keys+mask together is safe).

Layout strategy on device: activations are kept feature-major ("transposed",
features on partitions) so every projection consumes the previous output
directly; softmax runs on transposed scores S^T[k, q] with the mask folded
into the Exp bias (per-partition) and the row-sum obtained via an appended
ones-column on V.  Matmuls are bf16 with fp32 PSUM accumulation.

Host-side preprocessing (exact linear-algebra folds):
  - LN1 affine folded into wq/wk/wv;  1/sqrt(d_k) folded into wq
  - V bias folded into the O-projection bias (softmax rows sum to 1)
  - LN2 affine folded into w1
"""

import sys

sys.path.insert(0, "/opt/trn_rl_repo")

import numpy as np
import ml_dtypes

import concourse.bass as bass
import concourse.tile as tile
from concourse import bacc, mybir
from concourse.bass_utils import run_bass_kernel_spmd
from concourse.masks import make_identity

F32 = mybir.dt.float32
BF16 = mybir.dt.bfloat16
AF = mybir.ActivationFunctionType
ALU = mybir.AluOpType

D = 1024          # d_model
H = 16            # heads
DK = 64           # head dim
DFF = 4096        # ffn hidden
S = 2048          # keys per batch element (per core)
Q = 512           # query tokens per core
EPS = 1e-6
NCHIP = 8
VS = 68           # V slot stride per head (64 data + ones col + 3 pad, 4B aligned)


def _build():
    nc = bacc.Bacc("TRN2", target_bir_lowering=False, debug=False)

    x_all = nc.dram_tensor("x_all", [S, D], F32, kind="ExternalInput").ap()
    wq_d = nc.dram_tensor("wq_b", [D, D], BF16, kind="ExternalInput").ap()
    wk_d = nc.dram_tensor("wk_b", [D, D], BF16, kind="ExternalInput").ap()
    wv_d = nc.dram_tensor("wv_b", [D, D], BF16, kind="ExternalInput").ap()
    wo_d = nc.dram_tensor("wo_b", [D, D], BF16, kind="ExternalInput").ap()
    w1_d = nc.dram_tensor("w1_b", [D, DFF], BF16, kind="ExternalInput").ap()
    w2_d = nc.dram_tensor("w2_b", [DFF, D], BF16, kind="ExternalInput").ap()
    bq_d = nc.dram_tensor("bq_v", [D], F32, kind="ExternalInput").ap()
    bk_d = nc.dram_tensor("bk_v", [D], F32, kind="ExternalInput").ap()
    bo_d = nc.dram_tensor("bo_v", [D], F32, kind="ExternalInput").ap()
    b1_d = nc.dram_tensor("b1_v", [DFF], F32, kind="ExternalInput").ap()
    b2_d = nc.dram_tensor("b2_v", [D], F32, kind="ExternalInput").ap()
    mb_d = nc.dram_tensor("mb_v", [S], F32, kind="ExternalInput").ap()
    out_d = nc.dram_tensor("out", [Q, D], F32, kind="ExternalOutput").ap()

    with tile.TileContext(nc) as tc:
        _emit(nc, tc, x_all, wq_d, wk_d, wv_d, wo_d, w1_d, w2_d,
              bq_d, bk_d, bo_d, b1_d, b2_d, mb_d, out_d)
    nc.compile()
    return nc


def _emit(nc, tc, x_all, wq_d, wk_d, wv_d, wo_d, w1_d, w2_d,
          bq_d, bk_d, bo_d, b1_d, b2_d, mb_d, out_d):
    NT = S // 128          # 16 token tiles of the full batch element
    NQ = Q // 128          # 4 token tiles of own block
    NR = D // 128          # 8 feature chunks of d_model
    NF = DFF // 128        # 32 feature chunks of d_ff

    consts = tc.alloc_tile_pool(name="consts", bufs=1)
    ident_bf = consts.tile([128, 128], BF16)
    make_identity(nc, ident_bf)
    ident_f = consts.tile([128, 128], F32)
    make_identity(nc, ident_f)
    bq_sb = consts.tile([128, NR], F32)
    nc.sync.dma_start(bq_sb[:], bq_d.rearrange("(m p) -> p m", p=128))
    bk_sb = consts.tile([128, NR], F32)
    nc.sync.dma_start(bk_sb[:], bk_d.rearrange("(m p) -> p m", p=128))
    bo_sb = consts.tile([128, NR], F32)
    nc.sync.dma_start(bo_sb[:], bo_d.rearrange("(m p) -> p m", p=128))
    b1_sb = consts.tile([128, NF], F32)
    nc.sync.dma_start(b1_sb[:], b1_d.rearrange("(f p) -> p f", p=128))
    mb_sb = consts.tile([128, NT], F32)
    nc.sync.dma_start(mb_sb[:], mb_d.rearrange("(i p) -> p i", p=128))


    psum = tc.alloc_tile_pool(name="psum", bufs=1, space="PSUM")
    dram = tc.alloc_tile_pool(name="dram", bufs=1, space="DRAM")

    # ---- weight pools -------------------------------------------------
    poolW = tc.alloc_tile_pool(name="poolW", bufs=1, side="right")  # wq/wk/wv, phase A-B
    wq_sb = poolW.tile([128, NR * D], BF16)
    wk_sb = poolW.tile([128, NR * D], BF16)
    wv_sb = poolW.tile([128, NR * D], BF16)
    for r in range(NR):
        nc.sync.dma_start(wq_sb[:, r * D:(r + 1) * D], wq_d[r * 128:(r + 1) * 128, :])
        nc.sync.dma_start(wk_sb[:, r * D:(r + 1) * D], wk_d[r * 128:(r + 1) * 128, :])
        nc.sync.dma_start(wv_sb[:, r * D:(r + 1) * D], wv_d[r * 128:(r + 1) * 128, :])

    # ---- phase A: LN1 over all S tokens, transpose to feature-major ---
    poolA = tc.alloc_tile_pool(name="poolA", bufs=1, side="right")  # ln1T
    ln1T = poolA.tile([128, NR * S], BF16)               # chunk r at [r*S, +S)
    streamA = tc.alloc_tile_pool(name="streamA", bufs=3, side="right")
    for t in range(NT):
        xt = streamA.tile([128, D], F32, bufs=3)
        nc.sync.dma_start(xt[:], x_all[t * 128:(t + 1) * 128, :])
        stats = streamA.tile([128, 2, 6], F32, bufs=4)
        xg = xt.rearrange("p (g d) -> p g d", g=2)
        nc.vector.bn_stats(stats[:, 0, :], xg[:, 0, :])
        nc.vector.bn_stats(stats[:, 1, :], xg[:, 1, :])
        mv = streamA.tile([128, 2], F32, bufs=4)
        nc.vector.bn_aggr(mv[:], stats[:])
        rstd = streamA.tile([128, 1], F32, bufs=4)
        # std with Bessel correction (ddof=1), then 1/(std+eps)
        nc.scalar.activation(rstd[:], mv[:, 1:2], AF.Sqrt, scale=float(D) / (D - 1))
        nc.vector.tensor_scalar_add(rstd[:], rstd[:], EPS)
        nc.vector.reciprocal(rstd[:], rstd[:])
        lt = streamA.tile([128, D], BF16, bufs=3)
        nc.vector.tensor_scalar(
            out=lt[:], in0=xt[:], scalar1=mv[:, 0:1], scalar2=rstd[:],
            op0=ALU.subtract, op1=ALU.mult)
        for r in range(NR):
            tp = psum.tile([128, 128], BF16, tag="tr", bufs=2)
            nc.tensor.transpose(tp[:], lt[:, r * 128:(r + 1) * 128], ident_bf[:])
            nc.vector.tensor_copy(ln1T[:, r * S + t * 128: r * S + (t + 1) * 128], tp[:])
    streamA.release()

    # ---- phase B: Q^T, K^T (feature-major) and V (token-major) --------
    poolQKV = tc.alloc_tile_pool(name="poolQKV", bufs=1)
    QT = poolQKV.tile([128, NR * Q], BF16)        # chunk m at [m*Q, +Q)
    KT = poolQKV.tile([128, NR * S], BF16)        # chunk m at [m*S, +S)
    V_sb = poolQKV.tile([128, NT * 16 * VS], BF16)  # tok chunk t at [t*16*VS), head h at +h*VS
    Ou = poolQKV.tile([128, NR * Q], BF16)        # normalized attn out, QT layout
    # per-head softmax sums staged at 32-aligned partitions (DVE base-partition
    # rule), head h at (partition 32*(h//4), column block h%4)
    sums_st = poolQKV.tile([128, 4 * Q], F32)
    sums_all = poolQKV.tile([16, Q], F32)
    recipb = poolQKV.tile([16, Q], BF16)
    recip = poolQKV.tile([16, Q], F32)
    Vv = V_sb.rearrange("p (t h s) -> p t h s", t=NT, s=VS)
    nc.vector.memset(Vv[:, :, :, 64:65], 1.0)     # ones column for row-sums

    for m in range(NR):
        qps = psum.tile([128, Q], F32, tag="mm512", bufs=4)
        for r in range(NR):
            nc.tensor.matmul(
                qps[:], wq_sb[:, r * D + m * 128: r * D + (m + 1) * 128],
                ln1T[:, r * S: r * S + Q], start=(r == 0), stop=(r == NR - 1))
        nc.vector.tensor_scalar_add(QT[:, m * Q:(m + 1) * Q], qps[:], bq_sb[:, m:m + 1])
    for m in range(NR):
        for s4 in range(S // Q):
            kps = psum.tile([128, Q], F32, tag="mm512", bufs=4)
            for r in range(NR):
                nc.tensor.matmul(
                    kps[:], wk_sb[:, r * D + m * 128: r * D + (m + 1) * 128],
                    ln1T[:, r * S + s4 * Q: r * S + (s4 + 1) * Q],
                    start=(r == 0), stop=(r == NR - 1))
            nc.vector.tensor_scalar_add(
                KT[:, m * S + s4 * Q: m * S + (s4 + 1) * Q], kps[:], bk_sb[:, m:m + 1])
    for t in range(NT):
        for s2 in range(2):
            vps = psum.tile([128, Q], F32, tag="mm512", bufs=4)
            for r in range(NR):
                nc.tensor.matmul(
                    vps[:], ln1T[:, r * S + t * 128: r * S + (t + 1) * 128],
                    wv_sb[:, r * D + s2 * Q: r * D + (s2 + 1) * Q],
                    start=(r == 0), stop=(r == NR - 1))
            nc.vector.tensor_copy(
                Vv[:, t, s2 * 8:(s2 + 1) * 8, 0:64],
                vps.rearrange("p (h d) -> p h d", d=64))
    poolA.release()
    poolW.release()

    # ---- phase C: attention per head ---------------------------------
    poolD1 = tc.alloc_tile_pool(name="poolD1", bufs=1, side="right")  # x_own, mhaT, g
    x_own = poolD1.tile([128, NQ * D], F32)
    for t in range(NQ):
        nc.sync.dma_start(x_own[:, t * D:(t + 1) * D], x_all[t * 128:(t + 1) * 128, :])

    poolwo = tc.alloc_tile_pool(name="poolwo", bufs=1, side="right")
    wo_sb = poolwo.tile([128, NR * D], BF16)
    for r in range(NR):
        nc.sync.dma_start(wo_sb[:, r * D:(r + 1) * D], wo_d[r * 128:(r + 1) * 128, :])

    poolE = tc.alloc_tile_pool(name="poolE", bufs=1)
    E0 = poolE.tile([128, NT * Q], BF16)
    E1 = poolE.tile([128, NT * Q], BF16)

    for h in range(H):
        m, half = h // 2, (h % 2) * 64
        E = E0 if h % 2 == 0 else E1
        for i in range(NT):
            sps = psum.tile([128, Q], F32, tag="mm512", bufs=4)
            nc.tensor.matmul(
                sps[:], KT[half:half + 64, m * S + i * 128: m * S + (i + 1) * 128],
                QT[half:half + 64, m * Q:(m + 1) * Q], start=True, stop=True)
            nc.scalar.activation(
                E[:, i * Q:(i + 1) * Q], sps[:], AF.Exp,
                bias=mb_sb[:, i:i + 1], scale=1.0)
        ops = psum.tile([128, Q], F32, tag="av", bufs=2)
        for i in range(NT):
            nc.tensor.matmul(
                ops[0:65, :], V_sb[:, i * 16 * VS + h * VS: i * 16 * VS + h * VS + 65],
                E[:, i * Q:(i + 1) * Q], start=(i == 0), stop=(i == NT - 1))
        p4, c4 = 32 * (h // 4), (h % 4) * Q
        nc.vector.tensor_copy(sums_st[p4:p4 + 1, c4:c4 + Q], ops[64:65, :])
        nc.vector.tensor_copy(Ou[half:half + 64, m * Q:(m + 1) * Q], ops[0:64, :])

    poolE.release()
    sums_src = sums_st.rearrange("(a b) (h q) -> a b h q", b=32, h=4)[:, 0, :, :]
    nc.sync.dma_start(sums_all[:, :], sums_src)
    nc.vector.reciprocal(recip[:], sums_all[:])
    nc.vector.tensor_copy(recipb[:], recip[:])
    rb_d = dram.tile([16, Q], BF16)
    nc.sync.dma_start(rb_d[:], recipb[:])
    streamC = tc.alloc_tile_pool(name="streamC", bufs=2)
    for m in range(NR):     # head pair (2m, 2m+1) shares partition chunk m
        rb = streamC.tile([128, Q], BF16, bufs=2)
        nc.sync.dma_start(rb[0:64, :], rb_d[2 * m:2 * m + 1, :].to_broadcast([64, Q]))
        nc.sync.dma_start(rb[64:128, :], rb_d[2 * m + 1:2 * m + 2, :].to_broadcast([64, Q]))
        nc.vector.tensor_tensor(
            out=Ou[:, m * Q:(m + 1) * Q],
            in0=Ou[:, m * Q:(m + 1) * Q], in1=rb[:], op=ALU.mult)
    streamC.release()

    # ---- phase D: O-projection, residual, LN2, transpose g -----------
    mhaT = poolD1.tile([128, NR * Q], F32)
    for m in range(NR):
        mps = psum.tile([128, Q], F32, tag="mm512", bufs=4)
        for r in range(NR):
            nc.tensor.matmul(
                mps[:], wo_sb[:, r * D + m * 128: r * D + (m + 1) * 128],
                Ou[:, r * Q:(r + 1) * Q], start=(r == 0), stop=(r == NR - 1))
        nc.vector.tensor_scalar_add(mhaT[:, m * Q:(m + 1) * Q], mps[:], bo_sb[:, m:m + 1])
    poolwo.release()
    poolQKV.release()

    poolD2 = tc.alloc_tile_pool(name="poolD2", bufs=1)   # h, gT, b2b (live into E)
    h_sb = poolD2.tile([128, NQ * D], F32)
    gT = poolD2.tile([128, NR * Q], BF16)                # chunk r at [r*Q, +Q)
    b2b = poolD2.tile([128, D], F32)
    b2_bc = bass.AP(tensor=b2_d.tensor, offset=b2_d.offset, ap=[[0, 128]] + list(b2_d.ap))
    nc.sync.dma_start(b2b[:], b2_bc)

    for t in range(NQ):
        for r in range(NR):
            tp = psum.tile([128, 128], F32, tag="tr", bufs=2)
            nc.tensor.transpose(
                tp[:], mhaT[:, r * Q + t * 128: r * Q + (t + 1) * 128], ident_f[:])
            nc.vector.tensor_tensor(
                out=h_sb[:, t * D + r * 128: t * D + (r + 1) * 128],
                in0=tp[:], in1=x_own[:, t * D + r * 128: t * D + (r + 1) * 128],
                op=ALU.add)
    g_sb = poolD1.tile([128, NQ * D], BF16)
    streamD = tc.alloc_tile_pool(name="streamD", bufs=4, side="right")
    for t in range(NQ):
        ht = h_sb[:, t * D:(t + 1) * D]
        stats2 = streamD.tile([128, 2, 6], F32, bufs=4)
        hg = ht.rearrange("p (g d) -> p g d", g=2)
        nc.vector.bn_stats(stats2[:, 0, :], hg[:, 0, :])
        nc.vector.bn_stats(stats2[:, 1, :], hg[:, 1, :])
        mv2 = streamD.tile([128, 2], F32, bufs=4)
        nc.vector.bn_aggr(mv2[:], stats2[:])
        rstd2 = streamD.tile([128, 1], F32, bufs=4)
        nc.scalar.activation(rstd2[:], mv2[:, 1:2], AF.Sqrt, scale=float(D) / (D - 1))
        nc.vector.tensor_scalar_add(rstd2[:], rstd2[:], EPS)
        nc.vector.reciprocal(rstd2[:], rstd2[:])
        nc.vector.tensor_scalar(
            out=g_sb[:, t * D:(t + 1) * D], in0=ht, scalar1=mv2[:, 0:1],
            scalar2=rstd2[:], op0=ALU.subtract, op1=ALU.mult)
    streamD.release()
    for t in range(NQ):
        for r in range(NR):
            tg = psum.tile([128, 128], BF16, tag="tr", bufs=2)
            nc.tensor.transpose(
                tg[:], g_sb[:, t * D + r * 128: t * D + (r + 1) * 128], ident_bf[:])
            nc.vector.tensor_copy(gT[:, r * Q + t * 128: r * Q + (t + 1) * 128], tg[:])
    # h2 = h + b2 (after g extracted)
    for t in range(NQ):
        nc.vector.tensor_tensor(
            out=h_sb[:, t * D:(t + 1) * D], in0=h_sb[:, t * D:(t + 1) * D],
            in1=b2b[:], op=ALU.add)
    poolD1.release()

    # ---- phase E: FFN -------------------------------------------------
    poolF = tc.alloc_tile_pool(name="poolF", bufs=1)
    w1_sb = poolF.tile([128, NR * DFF], BF16)     # din chunk r at [r*DFF, +DFF)
    for r in range(NR):
        nc.sync.dma_start(w1_sb[:, r * DFF:(r + 1) * DFF], w1_d[r * 128:(r + 1) * 128, :])
    w2_sb = poolF.tile([128, NF * D], BF16)       # dff chunk f at [f*D, +D)
    for f in range(NF):
        nc.sync.dma_start(w2_sb[:, f * D:(f + 1) * D], w2_d[f * 128:(f + 1) * 128, :])
    H1T = poolF.tile([128, NF * Q], BF16)         # dff chunk f at [f*Q, +Q)

    for f in range(NF):
        fps = psum.tile([128, Q], F32, tag="mm512", bufs=4)
        for r in range(NR):
            nc.tensor.matmul(
                fps[:], w1_sb[:, r * DFF + f * 128: r * DFF + (f + 1) * 128],
                gT[:, r * Q:(r + 1) * Q], start=(r == 0), stop=(r == NR - 1))
        nc.scalar.activation(
            H1T[:, f * Q:(f + 1) * Q], fps[:], AF.Gelu, bias=b1_sb[:, f:f + 1], scale=1.0)

    streamE = tc.alloc_tile_pool(name="streamE", bufs=2)
    for t in range(NQ):
        for s2 in range(2):
            ops2 = psum.tile([128, Q], F32, tag="av", bufs=2)
            for f in range(NF):
                nc.tensor.matmul(
                    ops2[:], H1T[:, f * Q + t * 128: f * Q + (t + 1) * 128],
                    w2_sb[:, f * D + s2 * Q: f * D + (s2 + 1) * Q],
                    start=(f == 0), stop=(f == NF - 1))
            ot = streamE.tile([128, Q], F32, bufs=2)
            nc.vector.tensor_tensor(
                out=ot[:], in0=ops2[:],
                in1=h_sb[:, t * D + s2 * Q: t * D + (s2 + 1) * Q], op=ALU.add)
            nc.sync.dma_start(out_d[t * 128:(t + 1) * 128, s2 * Q:(s2 + 1) * Q], ot[:])
    streamE.release()
    poolF.release()
    poolD2.release()
    dram.release()
    consts.release()
    psum.release()


_NC = None


def _get_nc():
    global _NC
    if _NC is None:
        _NC = _build()
    return _NC


def _prep_in_maps(inputs):
    x = np.asarray(inputs["x"], np.float32)          # [2, 2048, 1024]
    mask = np.asarray(inputs["mask"])                # [2, 1, 1, 2048]
    wq, bq = np.asarray(inputs["wq"], np.float32), np.asarray(inputs["bq"], np.float32)
    wk, bk = np.asarray(inputs["wk"], np.float32), np.asarray(inputs["bk"], np.float32)
    wv, bv = np.asarray(inputs["wv"], np.float32), np.asarray(inputs["bv"], np.float32)
    wo, bo = np.asarray(inputs["wo"], np.float32), np.asarray(inputs["bo"], np.float32)
    ln1_w, ln1_b = np.asarray(inputs["ln1_w"], np.float32), np.asarray(inputs["ln1_b"], np.float32)
    ln2_w, ln2_b = np.asarray(inputs["ln2_w"], np.float32), np.asarray(inputs["ln2_b"], np.float32)
    w1, b1 = np.asarray(inputs["w1"], np.float32), np.asarray(inputs["b1"], np.float32)
    w2, b2 = np.asarray(inputs["w2"], np.float32), np.asarray(inputs["b2"], np.float32)

    bf = ml_dtypes.bfloat16
    sc = 1.0 / np.sqrt(np.float32(DK))
    wq_b = (ln1_w[:, None] * wq * sc).astype(bf)
    wk_b = (ln1_w[:, None] * wk).astype(bf)
    wv_b = (ln1_w[:, None] * wv).astype(bf)
    wo_b = wo.astype(bf)
    w1_b = (ln2_w[:, None] * w1).astype(bf)
    w2_b = w2.astype(bf)
    bq_v = ((ln1_b @ wq + bq) * sc).astype(np.float32)
    bk_v = (ln1_b @ wk + bk).astype(np.float32)
    bv_full = ln1_b @ wv + bv
    bo_v = (bv_full @ wo + bo).astype(np.float32)
    b1_v = (ln2_b @ w1 + b1).astype(np.float32)
    b2_v = b2.astype(np.float32)

    common = dict(wq_b=wq_b, wk_b=wk_b, wv_b=wv_b, wo_b=wo_b, w1_b=w1_b,
                  w2_b=w2_b, bq_v=bq_v, bk_v=bk_v, bo_v=bo_v, b1_v=b1_v,
                  b2_v=b2_v)
    in_maps = []
    for c in range(NCHIP):
        b, j = c // 4, c % 4
        q0 = j * Q
        xr = np.concatenate([x[b, q0:], x[b, :q0]], axis=0)
        mbias = np.where(mask[b, 0, 0] == 0, np.float32(-30000.0), np.float32(0.0))
        mbr = np.concatenate([mbias[q0:], mbias[:q0]]).astype(np.float32)
        in_maps.append(dict(common, x_all=np.ascontiguousarray(xr), mb_v=mbr))
    return in_maps


def kernel(**inputs):
    in_maps = _prep_in_maps(inputs)
    nc = _get_nc()
    res = run_bass_kernel_spmd(nc, in_maps, core_ids=list(range(NCHIP)))
    out = np.empty((2, S, D), np.float32)
    for c in range(NCHIP):
        b, j = c // 4, c % 4
        out[b, j * Q:(j + 1) * Q] = res.results[c]["out"]
    return out
